# revision 10
# baseline (speedup 1.0000x reference)
"""Trainium2 Bass kernel for the MemoryEfficientMambaBlock problem.

Data-parallel over 8 NeuronCores: x sharded over tokens, small weights
replicated. Per core, per 512-token tile:
  LayerNorm (bn_stats + magic-rsqrt + apply, all on DVE) -> PE transpose
  to feature-major bf16 -> ACT copyback (k-chunks 0-3 stay bf16,
  k-chunks 4-7 cast to fp8e4) -> matmul1 as a mixed-precision K-split:
  4 bf16 matmuls (K=128 each, exact-ish) + 2 fp8 DoubleRow matmuls
  (K=256 each) per 128-col output chunk, all accumulating in one PSUM
  bank at a common (x*8)(W*64) scaling -> SiLU per m-chunk with the
  1/512 descale fused -> bf16 matmul @W_stateT -> SiLU+(b_state+
  initial_state) -> K=9 f32r matmul (ones row carries b_out) with the
  residual add fused into the DVE copyback.

The K-split replaces the previous two-level fp8 scheme: on TRN2 a
DoubleRow fp8 matmul streams N=512 in ~216ns and a bf16 one in ~257ns
(DR only doubles K per instruction), so hi+lo fp8 cost equals plain
bf16 while being less accurate. Splitting K 4:4 between bf16 and
single-level fp8-DR cuts matmul1 from 8 to 6 instructions per m-chunk
and simultaneously improves rel err (1.61e-2 vs 1.79e-2 measured).

Scheduling: silu2 is emitted before the next tile's transpose
copybacks so matmul3 never queues behind them on ACT; matmul3 runs in
[P,512] chunks double-buffered over two 1-bank PSUM slots with the
next tile's transposes interleaved between chunks so the PE fills the
DVE-residual drain time; p2 shares the ps1 pool (4 bufs). Tile 0's
x-DMA/LayerNorm run in g-halves so the PE's first matmul starts ~8us
earlier; weight DMAs are split by m-range on the scalar queue.
"""

import sys

if "/opt/trn_rl_repo" not in sys.path:
    sys.path.insert(0, "/opt/trn_rl_repo")

import numpy as np
import ml_dtypes

import concourse.bass as bass
import concourse.mybir as mybir
import concourse.tile as tile
from concourse.bass_utils import run_bass_kernel_spmd

P = 128
D_MODEL = 1024
D_INNER = 2048
D_STATE = 8
EPS = 1e-5
N_CORES = 8
TOK_TOTAL = 2 * 128 * 196  # 50176
TOK = TOK_TOTAL // N_CORES  # 6272
TILE_T = 512
G = TILE_T // P  # 4

KD = D_MODEL // P  # 8 contraction chunks for matmul 1
KB = 3  # k-chunks 0..2 run in bf16
KF = KD - KB  # k-chunks 3..7 run in single-level fp8 DoubleRow
KP = (KF + 1) // 2  # DR pairs; the last pair is (chunk7, dead-slot*0)
ME = D_INNER // P  # 16 output tiles for matmul 1 / contraction chunks for 2

S_W = 64.0  # scale on W_proj (both bf16 and fp8 parts)
S_X = 8.0  # scale on xn (folded into rstd; both parts)

F32 = mybir.dt.float32
F32R = mybir.dt.float32r
BF16 = mybir.dt.bfloat16
FP8 = mybir.dt.float8e4
I32 = mybir.dt.int32

MAGIC = 0x5F3759DF


def _split_multi_waits(nc):
    """This container's walrus accepts at most ONE semaphore wait per
    instruction. Hoist all but the last wait of each instruction onto
    fresh single-wait NoOps inserted immediately before it on the same
    engine (the sequencer processes instructions in order, so semantics
    are unchanged)."""
    n_split = 0
    for f in nc.m.functions:
        for blk in f.blocks:
            out = []
            changed = False
            for inst in blk.instructions:
                si = inst.sync_info
                waits = list(si.on_wait) if si is not None else []
                if len(waits) > 1:
                    changed = True
                    for j, w in enumerate(waits[:-1]):
                        nop = mybir.InstNoOp(
                            name=f"{inst.name}-wsplit{j}", ins=[], outs=[]
                        )
                        nop.engine = inst.engine
                        nop.sync_info = mybir.SyncInfo(on_wait=[w], on_update=[])
                        out.append(nop)
                        n_split += 1
                    inst.sync_info = mybir.SyncInfo(
                        on_wait=[waits[-1]], on_update=list(si.on_update)
                    )
                out.append(inst)
            if changed:
                blk.instructions = out
    return n_split


def build_kernel(has_bias: bool):
    nc = bass.Bass()
    x = nc.dram_tensor("x", [TOK, D_MODEL], F32, kind="ExternalInput")
    # bf16 part of W_proj^T * S_W: k-chunks 0..3, [k*P+p, e]
    wbt = nc.dram_tensor("wbt", [KB * P, D_INNER], BF16, kind="ExternalInput")
    # fp8 part in DoubleRow pair layout [p, r, j, e]: chunk 4+2r+j
    wp8 = nc.dram_tensor("wp8", [P, KP, 2, D_INNER], FP8, kind="ExternalInput")
    wst = nc.dram_tensor("wst", [D_INNER, D_STATE], BF16, kind="ExternalInput")
    wo9 = nc.dram_tensor("wo9", [D_STATE + 1, D_MODEL], F32R, kind="ExternalInput")
    bp = nc.dram_tensor("bp", [1, D_INNER], F32R, kind="ExternalInput")
    b2 = nc.dram_tensor("b2", [D_STATE, 1], F32, kind="ExternalInput")
    ones = nc.dram_tensor("ones", [1, TILE_T], F32R, kind="ExternalInput")
    ident_d = nc.dram_tensor("ident", [P, P], BF16, kind="ExternalInput")
    y = nc.dram_tensor("y", [TOK, D_MODEL], F32, kind="ExternalOutput")

    # all tiles full-size; the last tile overlaps the previous one so the
    # matmuls always stream N=512
    tiles = [(o, TILE_T) for o in range(0, TOK - TILE_T + 1, TILE_T)]
    if tiles[-1][0] + TILE_T < TOK:
        tiles.append((TOK - TILE_T, TILE_T))

    with tile.TileContext(nc) as tc:
        with (
            tc.tile_pool(name="singles", bufs=1) as singles,
            tc.tile_pool(name="xpool", bufs=3) as xpool,
            tc.tile_pool(name="xnpool", bufs=2) as xnpool,
            tc.tile_pool(name="xtpool", bufs=2) as xtpool,
            tc.tile_pool(name="projp", bufs=2) as projp,
            tc.tile_pool(name="outp", bufs=2) as outp,
            tc.tile_pool(name="statp", bufs=6) as statp,
            tc.tile_pool(name="ps_tr", bufs=2, space="PSUM") as ps_tr,
            tc.tile_pool(name="ps1", bufs=3, space="PSUM") as ps1,
            tc.tile_pool(name="ps2", bufs=1, space="PSUM") as ps2,
            tc.tile_pool(name="ps3", bufs=2, space="PSUM") as ps3,
        ):
            # x tile 0 (in g-halves) and 1 fetched first on the sync queue
            # so LN/transpose work starts immediately; weights stream on
            # the scalar queue split by m-range so matmul m only waits
            # for its own slice.
            def a_dma(i, split=False):
                off, T = tiles[i]
                x_sb = xpool.tile([P, G, D_MODEL], F32, tag="x")
                src = x[off : off + T, :].rearrange("(g p) d -> p g d", p=P)
                if split:
                    for g in range(G):
                        nc.sync.dma_start(x_sb[:, g : g + 1], src[:, g : g + 1])
                else:
                    nc.sync.dma_start(x_sb, src)
                return x_sb

            x_tiles = [a_dma(0, split=True), a_dma(1)]

            # all weight/constant DMAs go on the sync queue: DMA issue
            # occupies the issuing engine's instruction stream, and the
            # ACT engine must run the transpose casts ASAP at startup
            ident = singles.tile([P, P], BF16)
            nc.sync.dma_start(ident, ident_d[:, :])
            b2_sb = singles.tile([D_STATE, 1], F32)
            nc.sync.dma_start(b2_sb, b2[:, :])
            wbt_sb = singles.tile([P, KB, D_INNER], BF16)
            wp8_sb = singles.tile([P, KP, 2, D_INNER], FP8)
            wbt_r = wbt[:, :].rearrange("(k p) e -> p k e", p=P)
            MR = D_INNER // 8  # 256-wide m-ranges
            for j in range(8):
                nc.sync.dma_start(
                    wbt_sb[:, :, j * MR : (j + 1) * MR],
                    wbt_r[:, :, j * MR : (j + 1) * MR],
                )
                nc.sync.dma_start(
                    wp8_sb[:, :, :, j * MR : (j + 1) * MR],
                    wp8[:, :, :, j * MR : (j + 1) * MR],
                )
            wst_sb = singles.tile([P, ME, D_STATE], BF16)
            nc.sync.dma_start(wst_sb, wst[:, :].rearrange("(k p) s -> p k s", p=P))
            wo9_sb = singles.tile([D_STATE + 1, D_MODEL], F32R)
            nc.sync.dma_start(wo9_sb, wo9[:, :])
            if has_bias:
                bp_sb = singles.tile([1, D_INNER], F32R)
                nc.sync.dma_start(bp_sb, bp[:, :])
                ones_sb = singles.tile([1, TILE_T], F32R)
                nc.sync.dma_start(ones_sb, ones[:, :])

            def a_ln_half(x_sb, xn_sb, half, tagsuf):
                """layernorm chain for g in one half -> xn (token-major
                bf16, scaled by S_X via rstd = rsqrt((var+eps)/S_X^2))"""
                GH = G // 2
                g0 = half * GH
                stats = statp.tile([P, GH, 2, 6], F32, tag="bnst" + tagsuf)
                mv = statp.tile([P, GH, 2], F32, tag="mv" + tagsuf)
                for gg in range(GH):
                    g = g0 + gg
                    nc.vector.bn_stats(stats[:, gg, 0, :], x_sb[:, g, 0:512])
                    nc.vector.bn_stats(stats[:, gg, 1, :], x_sb[:, g, 512:1024])
                    nc.vector.bn_aggr(mv[:, gg, :], stats[:, gg])
                vp = statp.tile([P, GH], F32, tag="vp" + tagsuf)
                nc.vector.tensor_scalar(
                    out=vp,
                    in0=mv[:, :, 1],
                    scalar1=EPS,
                    scalar2=1.0 / (S_X * S_X),
                    op0=mybir.AluOpType.add,
                    op1=mybir.AluOpType.mult,
                )
                # magic rsqrt + two Newton steps (rel err ~5e-6)
                rs = statp.tile([P, GH], F32, tag="rs" + tagsuf)
                nc.vector.tensor_scalar(
                    out=rs.bitcast(I32),
                    in0=vp.bitcast(I32),
                    scalar1=1,
                    scalar2=None,
                    op0=mybir.AluOpType.arith_shift_right,
                )
                nc.vector.tensor_scalar(
                    out=rs.bitcast(I32),
                    in0=rs.bitcast(I32),
                    scalar1=-1,
                    scalar2=MAGIC,
                    op0=mybir.AluOpType.mult,
                    op1=mybir.AluOpType.add,
                )
                sq = statp.tile([P, GH], F32, tag="sq" + tagsuf)
                for _ in range(2):
                    nc.vector.tensor_tensor(
                        out=sq, in0=rs, in1=rs, op=mybir.AluOpType.mult
                    )
                    nc.vector.tensor_tensor(
                        out=sq, in0=sq, in1=vp, op=mybir.AluOpType.mult
                    )
                    nc.vector.tensor_scalar(
                        out=sq,
                        in0=sq,
                        scalar1=-0.5,
                        scalar2=1.5,
                        op0=mybir.AluOpType.mult,
                        op1=mybir.AluOpType.add,
                    )
                    nc.vector.tensor_tensor(
                        out=rs, in0=rs, in1=sq, op=mybir.AluOpType.mult
                    )
                for gg in range(GH):
                    g = g0 + gg
                    nc.vector.tensor_scalar(
                        out=xn_sb[:, g, :],
                        in0=x_sb[:, g, :],
                        scalar1=mv[:, gg, 0:1],
                        scalar2=rs[:, gg : gg + 1],
                        op0=mybir.AluOpType.subtract,
                        op1=mybir.AluOpType.mult,
                    )

            def a_ln(x_sb, split=False):
                xn_sb = xnpool.tile([P, G, D_MODEL], BF16, tag="xn")
                if split:
                    a_ln_half(x_sb, xn_sb, 0, "a")
                    a_ln_half(x_sb, xn_sb, 1, "b")
                else:
                    # single chain over all G (fewer small ops)
                    stats = statp.tile([P, G, 2, 6], F32, tag="bnst")
                    mv = statp.tile([P, G, 2], F32, tag="mv")
                    for g in range(G):
                        nc.vector.bn_stats(stats[:, g, 0, :], x_sb[:, g, 0:512])
                        nc.vector.bn_stats(stats[:, g, 1, :], x_sb[:, g, 512:1024])
                        nc.vector.bn_aggr(mv[:, g, :], stats[:, g])
                    vp = statp.tile([P, G], F32, tag="vp")
                    nc.vector.tensor_scalar(
                        out=vp,
                        in0=mv[:, :, 1],
                        scalar1=EPS,
                        scalar2=1.0 / (S_X * S_X),
                        op0=mybir.AluOpType.add,
                        op1=mybir.AluOpType.mult,
                    )
                    rs = statp.tile([P, G], F32, tag="rs")
                    nc.vector.tensor_scalar(
                        out=rs.bitcast(I32),
                        in0=vp.bitcast(I32),
                        scalar1=1,
                        scalar2=None,
                        op0=mybir.AluOpType.arith_shift_right,
                    )
                    nc.vector.tensor_scalar(
                        out=rs.bitcast(I32),
                        in0=rs.bitcast(I32),
                        scalar1=-1,
                        scalar2=MAGIC,
                        op0=mybir.AluOpType.mult,
                        op1=mybir.AluOpType.add,
                    )
                    sq = statp.tile([P, G], F32, tag="sq")
                    for _ in range(2):
                        nc.vector.tensor_tensor(
                            out=sq, in0=rs, in1=rs, op=mybir.AluOpType.mult
                        )
                        nc.vector.tensor_tensor(
                            out=sq, in0=sq, in1=vp, op=mybir.AluOpType.mult
                        )
                        nc.vector.tensor_scalar(
                            out=sq,
                            in0=sq,
                            scalar1=-0.5,
                            scalar2=1.5,
                            op0=mybir.AluOpType.mult,
                            op1=mybir.AluOpType.add,
                        )
                        nc.vector.tensor_tensor(
                            out=rs, in0=rs, in1=sq, op=mybir.AluOpType.mult
                        )
                    for g in range(G):
                        nc.vector.tensor_scalar(
                            out=xn_sb[:, g, :],
                            in0=x_sb[:, g, :],
                            scalar1=mv[:, g, 0:1],
                            scalar2=rs[:, g : g + 1],
                            op0=mybir.AluOpType.subtract,
                            op1=mybir.AluOpType.mult,
                        )
                return xn_sb

            def tr_alloc():
                xtb = xtpool.tile([P, KB, G, P], BF16, tag="xtb")
                xt8 = xtpool.tile([P, KF + 1, G, P], FP8, tag="xt8")
                if KF % 2:
                    # dead slot pairs with chunk 7 in the last DR matmul
                    # (zero weights); gpsimd is otherwise idle
                    nc.gpsimd.memset(xt8[:, KF : KF + 1], 0)
                return xtb, xt8

            def tr_quarter(xt, xn_sb, r, half):
                """transposes for pair r (k-chunks 2r, 2r+1), g-half."""
                xtb, xt8, ptrs = xt
                if half == 0:
                    ptr = ps_tr.tile([P, 2, G, P], BF16, tag="ptr")
                    ptrs[r] = ptr
                ptr = ptrs[r]
                gs = range(half * (G // 2), (half + 1) * (G // 2))
                for kk in range(2):
                    k = 2 * r + kk
                    for g in gs:
                        nc.tensor.transpose(
                            ptr[:, kk, g, :],
                            xn_sb[:, g, k * P : (k + 1) * P],
                            ident,
                        )

            def tr_cast(xt, r, half=None):
                """ACT copyback for pair r (optionally one g-half).
                Chunk c goes to xtb[c] (c < KB) or xt8[c-KB]; pair 1
                straddles the split so it issues two half-width copies.
                The dead slot xt8[KF] is filled with chunk-7 data (its
                weights are zero)."""
                xtb, xt8, ptrs = xt
                ptr = ptrs[r]
                if half is None:
                    gsl = slice(0, G)
                else:
                    gsl = slice(half * (G // 2), (half + 1) * (G // 2))
                c0 = 2 * r
                if c0 + 1 < KB:
                    nc.scalar.activation(
                        out=xtb[:, c0 : c0 + 2, gsl],
                        in_=ptr[:, :, gsl],
                        func=mybir.ActivationFunctionType.Copy,
                    )
                elif c0 >= KB:
                    nc.scalar.activation(
                        out=xt8[:, c0 - KB : c0 - KB + 2, gsl],
                        in_=ptr[:, :, gsl],
                        func=mybir.ActivationFunctionType.Copy,
                    )
                else:
                    nc.scalar.activation(
                        out=xtb[:, c0 : c0 + 1, gsl],
                        in_=ptr[:, 0:1, gsl],
                        func=mybir.ActivationFunctionType.Copy,
                    )
                    nc.scalar.activation(
                        out=xt8[:, 0:1, gsl],
                        in_=ptr[:, 1:2, gsl],
                        func=mybir.ActivationFunctionType.Copy,
                    )

            def a_tr_all(xn_sb, split=False):
                """full transpose+cast for one tile (startup path)"""
                xtb, xt8 = tr_alloc()
                xt = (xtb, xt8, {})
                for r in range(KD // 2):
                    if split:
                        tr_quarter(xt, xn_sb, r, 0)
                        tr_cast(xt, r, 0)
                        tr_quarter(xt, xn_sb, r, 1)
                        tr_cast(xt, r, 1)
                    else:
                        tr_quarter(xt, xn_sb, r, 0)
                        tr_quarter(xt, xn_sb, r, 1)
                        tr_cast(xt, r)
                return xtb, xt8

            # software pipeline: x-DMA two tiles ahead, LayerNorm one tile
            # ahead, transposes one tile ahead interleaved into the m3
            # chunk loop
            xn_cur = a_ln(x_tiles[0], split=True)
            xt_cur = a_tr_all(xn_cur, split=True)
            xn_next = a_ln(x_tiles[1])
            for i, (off, T) in enumerate(tiles):
                x_sb = x_tiles[i]
                xtb, xt8 = xt_cur
                if i + 2 < len(tiles):
                    x_tiles.append(a_dma(i + 2))
                # the final overlapped tile recomputes only its genuinely
                # new tokens (the trailing g-groups) through the matmuls;
                # LN/transpose of the overlap region is off critical path
                is_ov = i == len(tiles) - 1 and TOK % TILE_T != 0
                g_lo = (TILE_T - TOK % TILE_T) // P if is_ov else 0
                nT = T - g_lo * P
                # cs9 allocated + ones row DMA'd early (row 8 is only
                # reachable by DMA; issuing here hides its latency)
                cs9 = statp.tile([D_STATE + 1, TILE_T], F32R, tag="cs9")
                nc.sync.dma_start(
                    cs9[D_STATE : D_STATE + 1, :nT], ones[:, :nT]
                )
                # matmul 1: K-split, common (x*S_X)(W*S_W) scaling in PSUM;
                # SiLU per m-chunk with the 1/(S_W*S_X) descale fused
                projT = projp.tile([P, ME, TILE_T], BF16, tag="projT")
                for m in range(ME):
                    p1 = ps1.tile([P, TILE_T], F32, tag="p1")
                    if has_bias:
                        nc.tensor.matmul(
                            p1[:, :nT],
                            lhsT=bp_sb[:, m * P : (m + 1) * P],
                            rhs=ones_sb[:, :nT],
                            start=True,
                            stop=False,
                            skip_group_check=True,
                        )
                    for k in range(KB):
                        nc.tensor.matmul(
                            p1[:, :nT],
                            lhsT=wbt_sb[:, k, m * P : (m + 1) * P],
                            rhs=xtb[:, k, g_lo:, :],
                            start=(k == 0 and not has_bias),
                            stop=False,
                            skip_group_check=has_bias,
                        )
                    for r in range(KP):
                        nc.tensor.matmul(
                            p1[:, :nT],
                            lhsT=wp8_sb[:, r, :, m * P : (m + 1) * P],
                            rhs=xt8[:, 2 * r : 2 * r + 2, g_lo:, :],
                            start=False,
                            stop=(r == KP - 1),
                            perf_mode=mybir.MatmulPerfMode.DoubleRow,
                            skip_group_check=has_bias,
                        )
                    nc.scalar.activation(
                        out=projT[:, m, :nT],
                        in_=p1[:, :nT],
                        func=mybir.ActivationFunctionType.Silu,
                        bias=0.0,
                        scale=1.0 / (S_W * S_X),
                    )
                # matmul 2: bf16, [D_STATE, T]
                p2 = ps2.tile([D_STATE, TILE_T], F32, tag="p2")
                for k2 in range(ME):
                    nc.tensor.matmul(
                        p2[:, :nT],
                        lhsT=wst_sb[:, k2, :],
                        rhs=projT[:, k2, :nT],
                        start=(k2 == 0),
                        stop=(k2 == ME - 1),
                    )
                # silu2 emitted BEFORE the next tile's transpose casts so
                # matmul 3 never queues behind them on the ACT engine
                nc.scalar.activation(
                    out=cs9[:D_STATE, :nT],
                    in_=p2[:, :nT],
                    func=mybir.ActivationFunctionType.Silu,
                    bias=b2_sb,
                    scale=1.0,
                )
                # matmul 3 in [P,512] chunks (double-buffered over two
                # 1-bank PSUM slots), with the next tile's transposes
                # interleaved so the PE fills the DVE-residual drain time;
                # residual add fused into the DVE copyback
                do_tr = i + 1 < len(tiles)
                if do_tr:
                    xtb_n, xt8_n = tr_alloc()
                    xt_n = (xtb_n, xt8_n, {})
                out_sb = outp.tile([P, G, D_MODEL], F32, tag="out")
                nch = (G - g_lo) * 2
                for c in range(nch):
                    g = g_lo + c // 2
                    h = c % 2
                    p3 = ps3.tile([P, 512], F32, tag="p3")
                    nc.tensor.matmul(
                        p3[:, :],
                        lhsT=cs9[:, (g - g_lo) * P : (g - g_lo + 1) * P],
                        rhs=wo9_sb[:, h * 512 : (h + 1) * 512],
                        start=True,
                        stop=True,
                    )
                    # interleave one transpose quarter per chunk
                    if do_tr and c < 8:
                        r, half = c // 2, c % 2
                        tr_quarter(xt_n, xn_next, r, half)
                        if half == 1:
                            tr_cast(xt_n, r)
                    nc.vector.tensor_add(
                        out=out_sb[:, g, h * 512 : (h + 1) * 512],
                        in0=p3[:, :],
                        in1=x_sb[:, g, h * 512 : (h + 1) * 512],
                    )
                if do_tr:
                    # finish any transpose quarters not covered (nch < 8)
                    for c in range(nch, 8):
                        r, half = c // 2, c % 2
                        tr_quarter(xt_n, xn_next, r, half)
                        if half == 1:
                            tr_cast(xt_n, r)
                    xt_cur = (xtb_n, xt8_n)
                nc.sync.dma_start(
                    y[off + g_lo * P : off + T, :].rearrange(
                        "(g p) d -> p g d", p=P
                    ),
                    out_sb[:, g_lo:, :],
                )
                # LN for the tile after is emitted BEHIND this tile's
                # residual adds: the DVE queue is in-order, and parking
                # ~10us of LN work ahead of the resid TTs would stall
                # matmul 3 on the ps3 rotation
                if i + 2 < len(tiles):
                    xn_next = a_ln(x_tiles[i + 2])

    _split_multi_waits(nc)
    return nc


_NC_CACHE = {}


def _get_nc(has_bias: bool):
    if has_bias not in _NC_CACHE:
        _NC_CACHE[has_bias] = build_kernel(has_bias)
    return _NC_CACHE[has_bias]


def make_in_maps(inputs):
    x = np.ascontiguousarray(inputs["x"], dtype=np.float32).reshape(-1, D_MODEL)
    W_proj = np.asarray(inputs["W_proj"], dtype=np.float64)
    b_proj = np.asarray(inputs["b_proj"], dtype=np.float64)
    W_state = np.asarray(inputs["W_state"], dtype=np.float32)
    b_state = np.asarray(inputs["b_state"], dtype=np.float32)
    W_out = np.asarray(inputs["W_out"], dtype=np.float32)
    b_out = np.asarray(inputs["b_out"], dtype=np.float32)
    initial_state = np.asarray(inputs["initial_state"], dtype=np.float32)
    gamma = np.asarray(inputs["gamma"], dtype=np.float64)
    beta = np.asarray(inputs["beta"], dtype=np.float64)

    # fold the LayerNorm affine into the projection
    Wp = W_proj * gamma[None, :]  # [d_inner, d_model]
    bp = b_proj + W_proj @ beta  # [d_inner]
    has_bias = bool(np.any(bp != 0.0))

    WpT = np.ascontiguousarray(Wp.T) * S_W  # [d_model, d_inner], scaled
    # k-chunks 0..3 in bf16
    wbt = WpT[: KB * P].astype(ml_dtypes.bfloat16)
    # k-chunks KB..7 in fp8e4, DoubleRow pair layout [p, r, j, e]; the
    # last pair's second slot is zero weights (dead xt8 slot)
    w8 = np.clip(WpT[KB * P :], -224.0, 224.0).astype(ml_dtypes.float8_e4m3)
    w8full = np.zeros((KP * 2, P, D_INNER), dtype=ml_dtypes.float8_e4m3)
    w8full[:KF] = w8.reshape(KF, P, D_INNER)
    wp8 = np.ascontiguousarray(
        w8full.reshape(KP, 2, P, D_INNER).transpose(2, 0, 1, 3)
    )

    shared = {
        "wbt": np.ascontiguousarray(wbt),
        "wp8": wp8,
        "wst": np.ascontiguousarray(W_state.T.astype(ml_dtypes.bfloat16)),
        "wo9": np.ascontiguousarray(
            np.concatenate([W_out.T, b_out[None, :]], axis=0)
        ),
        "bp": np.ascontiguousarray((bp * S_W * S_X).astype(np.float32))[None, :],
        "b2": np.ascontiguousarray(
            (b_state + initial_state.reshape(-1)).reshape(D_STATE, 1)
        ),
        "ones": np.ones((1, TILE_T), dtype=np.float32),
        "ident": np.eye(P, dtype=ml_dtypes.bfloat16),
    }
    in_maps = []
    for c in range(N_CORES):
        m = {"x": np.ascontiguousarray(x[c * TOK : (c + 1) * TOK])}
        m.update(shared)
        in_maps.append(m)
    return in_maps, has_bias


def kernel(**inputs) -> np.ndarray:
    in_maps, has_bias = make_in_maps(inputs)
    nc = _get_nc(has_bias)
    res = run_bass_kernel_spmd(nc, in_maps, core_ids=list(range(N_CORES)))
    out = np.concatenate([res.results[c]["y"] for c in range(N_CORES)], axis=0)
    return out.reshape(np.asarray(inputs["x"]).shape)


# revision 13
# speedup vs baseline: 1.1168x; 1.1168x over previous
"""Trainium2 Bass kernel for the MemoryEfficientMambaBlock problem.

Data-parallel over 8 NeuronCores: x sharded over tokens, small weights
replicated. Per core, per 512-token tile:
  LayerNorm (bn_stats + magic-rsqrt + apply, all on DVE) -> PE transpose
  to feature-major bf16 -> ACT copyback (k-chunks 0-3 stay bf16,
  k-chunks 4-7 cast to fp8e4) -> matmul1 as a mixed-precision K-split:
  4 bf16 matmuls (K=128 each, exact-ish) + 2 fp8 DoubleRow matmuls
  (K=256 each) per 128-col output chunk, all accumulating in one PSUM
  bank at a common (x*8)(W*64) scaling -> SiLU per m-chunk with the
  1/512 descale fused -> bf16 matmul @W_stateT -> SiLU+(b_state+
  initial_state) -> K=9 f32r matmul (ones row carries b_out) with the
  residual add fused into the DVE copyback.

The K-split replaces the previous two-level fp8 scheme: on TRN2 a
DoubleRow fp8 matmul streams N=512 in ~216ns and a bf16 one in ~257ns
(DR only doubles K per instruction), so hi+lo fp8 cost equals plain
bf16 while being less accurate. Splitting K 4:4 between bf16 and
single-level fp8-DR cuts matmul1 from 8 to 6 instructions per m-chunk
and simultaneously improves rel err (1.61e-2 vs 1.79e-2 measured).

Scheduling: silu2 is emitted before the next tile's transpose
copybacks so matmul3 never queues behind them on ACT; matmul3 runs in
[P,512] chunks double-buffered over two 1-bank PSUM slots with the
next tile's transposes interleaved between chunks so the PE fills the
DVE-residual drain time; p2 shares the ps1 pool (4 bufs). Tile 0's
x-DMA/LayerNorm run in g-halves so the PE's first matmul starts ~8us
earlier; weight DMAs are split by m-range on the scalar queue.
"""

import sys

if "/opt/trn_rl_repo" not in sys.path:
    sys.path.insert(0, "/opt/trn_rl_repo")

import numpy as np
import ml_dtypes

import concourse.bass as bass
import concourse.mybir as mybir
import concourse.tile as tile
from concourse.bass_utils import run_bass_kernel_spmd

P = 128
D_MODEL = 1024
D_INNER = 2048
D_STATE = 8
EPS = 1e-5
N_CORES = 8
TOK_TOTAL = 2 * 128 * 196  # 50176
TOK = TOK_TOTAL // N_CORES  # 6272
TILE_T = 512
G = TILE_T // P  # 4

KD = D_MODEL // P  # 8 contraction chunks for matmul 1
KB = 4  # k-chunks 0..3 run in bf16
KF = KD - KB  # k-chunks 4..7 run in single-level fp8 DoubleRow
KP = (KF + 1) // 2  # DR pairs (odd KF would pair chunk 7 with a zero-weight dead slot)
ME = D_INNER // P  # 16 output tiles for matmul 1 / contraction chunks for 2

S_W = 64.0  # scale on W_proj (both bf16 and fp8 parts)
S_X = 8.0  # scale on xn (folded into rstd; both parts)

F32 = mybir.dt.float32
F32R = mybir.dt.float32r
BF16 = mybir.dt.bfloat16
FP8 = mybir.dt.float8e4
I32 = mybir.dt.int32

MAGIC = 0x5F3759DF


def _split_multi_waits(nc):
    """This container's walrus accepts at most ONE semaphore wait per
    instruction. Hoist all but the last wait of each instruction onto
    fresh single-wait NoOps inserted immediately before it on the same
    engine (the sequencer processes instructions in order, so semantics
    are unchanged)."""
    n_split = 0
    for f in nc.m.functions:
        for blk in f.blocks:
            out = []
            changed = False
            for inst in blk.instructions:
                si = inst.sync_info
                waits = list(si.on_wait) if si is not None else []
                if len(waits) > 1:
                    changed = True
                    for j, w in enumerate(waits[:-1]):
                        nop = mybir.InstNoOp(
                            name=f"{inst.name}-wsplit{j}", ins=[], outs=[]
                        )
                        nop.engine = inst.engine
                        nop.sync_info = mybir.SyncInfo(on_wait=[w], on_update=[])
                        out.append(nop)
                        n_split += 1
                    inst.sync_info = mybir.SyncInfo(
                        on_wait=[waits[-1]], on_update=list(si.on_update)
                    )
                out.append(inst)
            if changed:
                blk.instructions = out
    return n_split


def build_kernel(has_bias: bool):
    nc = bass.Bass()
    x = nc.dram_tensor("x", [TOK, D_MODEL], F32, kind="ExternalInput")
    # bf16 part of W_proj^T * S_W: k-chunks 0..3, [k*P+p, e]
    wbt = nc.dram_tensor("wbt", [KB * P, D_INNER], BF16, kind="ExternalInput")
    # fp8 part in DoubleRow pair layout [p, r, j, e]: chunk 4+2r+j
    wp8 = nc.dram_tensor("wp8", [P, KP, 2, D_INNER], FP8, kind="ExternalInput")
    wst = nc.dram_tensor("wst", [D_INNER, D_STATE], BF16, kind="ExternalInput")
    wo9 = nc.dram_tensor("wo9", [D_STATE + 1, D_MODEL], F32R, kind="ExternalInput")
    bp = nc.dram_tensor("bp", [1, D_INNER], F32R, kind="ExternalInput")
    b2 = nc.dram_tensor("b2", [D_STATE, 1], F32, kind="ExternalInput")
    ones = nc.dram_tensor("ones", [1, TILE_T], F32R, kind="ExternalInput")
    ident_d = nc.dram_tensor("ident", [P, P], BF16, kind="ExternalInput")
    y = nc.dram_tensor("y", [TOK, D_MODEL], F32, kind="ExternalOutput")

    # all tiles full-size; the last tile overlaps the previous one so the
    # matmuls always stream N=512
    tiles = [(o, TILE_T) for o in range(0, TOK - TILE_T + 1, TILE_T)]
    if tiles[-1][0] + TILE_T < TOK:
        tiles.append((TOK - TILE_T, TILE_T))

    with tile.TileContext(nc) as tc:
        with (
            tc.tile_pool(name="singles", bufs=1) as singles,
            tc.tile_pool(name="xpool", bufs=3) as xpool,
            tc.tile_pool(name="xnpool", bufs=2) as xnpool,
            tc.tile_pool(name="xtpool", bufs=2) as xtpool,
            tc.tile_pool(name="projp", bufs=2) as projp,
            tc.tile_pool(name="outp", bufs=2) as outp,
            tc.tile_pool(name="statp", bufs=6) as statp,
            tc.tile_pool(name="ps_tr", bufs=2, space="PSUM") as ps_tr,
            tc.tile_pool(name="ps1", bufs=3, space="PSUM") as ps1,
            tc.tile_pool(name="ps2", bufs=1, space="PSUM") as ps2,
            tc.tile_pool(name="ps3", bufs=2, space="PSUM") as ps3,
        ):
            # x tile 0 (in g-halves) and 1 fetched first on the sync queue
            # so LN/transpose work starts immediately; weights stream on
            # the scalar queue split by m-range so matmul m only waits
            # for its own slice.
            def a_dma(i, split=False):
                off, T = tiles[i]
                x_sb = xpool.tile([P, G, D_MODEL], F32, tag="x")
                src = x[off : off + T, :].rearrange("(g p) d -> p g d", p=P)
                if split:
                    for g in range(G):
                        nc.sync.dma_start(x_sb[:, g : g + 1], src[:, g : g + 1])
                else:
                    nc.sync.dma_start(x_sb, src)
                return x_sb

            x_tiles = [a_dma(0, split=True), a_dma(1)]

            # all weight/constant DMAs go on the sync queue: DMA issue
            # occupies the issuing engine's instruction stream, and the
            # ACT engine must run the transpose casts ASAP at startup
            ident = singles.tile([P, P], BF16)
            nc.sync.dma_start(ident, ident_d[:, :])
            b2_sb = singles.tile([D_STATE, 1], F32)
            nc.sync.dma_start(b2_sb, b2[:, :])
            wbt_sb = singles.tile([P, KB, D_INNER], BF16)
            wp8_sb = singles.tile([P, KP, 2, D_INNER], FP8)
            wbt_r = wbt[:, :].rearrange("(k p) e -> p k e", p=P)
            MR = D_INNER // 8  # 256-wide m-ranges
            for j in range(8):
                nc.sync.dma_start(
                    wbt_sb[:, :, j * MR : (j + 1) * MR],
                    wbt_r[:, :, j * MR : (j + 1) * MR],
                )
                nc.sync.dma_start(
                    wp8_sb[:, :, :, j * MR : (j + 1) * MR],
                    wp8[:, :, :, j * MR : (j + 1) * MR],
                )
            wst_sb = singles.tile([P, ME, D_STATE], BF16)
            nc.sync.dma_start(wst_sb, wst[:, :].rearrange("(k p) s -> p k s", p=P))
            wo9_sb = singles.tile([D_STATE + 1, D_MODEL], F32R)
            nc.sync.dma_start(wo9_sb, wo9[:, :])
            if has_bias:
                bp_sb = singles.tile([1, D_INNER], F32R)
                nc.sync.dma_start(bp_sb, bp[:, :])
                ones_sb = singles.tile([1, TILE_T], F32R)
                nc.sync.dma_start(ones_sb, ones[:, :])

            def a_ln_half(x_sb, xn_sb, half, tagsuf):
                """layernorm chain for g in one half -> xn (token-major
                bf16, scaled by S_X via rstd = rsqrt((var+eps)/S_X^2))"""
                GH = G // 2
                g0 = half * GH
                stats = statp.tile([P, GH, 2, 6], F32, tag="bnst" + tagsuf)
                mv = statp.tile([P, GH, 2], F32, tag="mv" + tagsuf)
                for gg in range(GH):
                    g = g0 + gg
                    nc.vector.bn_stats(stats[:, gg, 0, :], x_sb[:, g, 0:512])
                    nc.vector.bn_stats(stats[:, gg, 1, :], x_sb[:, g, 512:1024])
                    nc.vector.bn_aggr(mv[:, gg, :], stats[:, gg])
                vp = statp.tile([P, GH], F32, tag="vp" + tagsuf)
                nc.vector.tensor_scalar(
                    out=vp,
                    in0=mv[:, :, 1],
                    scalar1=EPS,
                    scalar2=1.0 / (S_X * S_X),
                    op0=mybir.AluOpType.add,
                    op1=mybir.AluOpType.mult,
                )
                # magic rsqrt + two Newton steps (rel err ~5e-6)
                rs = statp.tile([P, GH], F32, tag="rs" + tagsuf)
                nc.vector.tensor_scalar(
                    out=rs.bitcast(I32),
                    in0=vp.bitcast(I32),
                    scalar1=1,
                    scalar2=None,
                    op0=mybir.AluOpType.arith_shift_right,
                )
                nc.vector.tensor_scalar(
                    out=rs.bitcast(I32),
                    in0=rs.bitcast(I32),
                    scalar1=-1,
                    scalar2=MAGIC,
                    op0=mybir.AluOpType.mult,
                    op1=mybir.AluOpType.add,
                )
                sq = statp.tile([P, GH], F32, tag="sq" + tagsuf)
                for _ in range(2):
                    nc.vector.tensor_tensor(
                        out=sq, in0=rs, in1=rs, op=mybir.AluOpType.mult
                    )
                    nc.vector.tensor_tensor(
                        out=sq, in0=sq, in1=vp, op=mybir.AluOpType.mult
                    )
                    nc.vector.tensor_scalar(
                        out=sq,
                        in0=sq,
                        scalar1=-0.5,
                        scalar2=1.5,
                        op0=mybir.AluOpType.mult,
                        op1=mybir.AluOpType.add,
                    )
                    nc.vector.tensor_tensor(
                        out=rs, in0=rs, in1=sq, op=mybir.AluOpType.mult
                    )
                for gg in range(GH):
                    g = g0 + gg
                    nc.vector.tensor_scalar(
                        out=xn_sb[:, g, :],
                        in0=x_sb[:, g, :],
                        scalar1=mv[:, gg, 0:1],
                        scalar2=rs[:, gg : gg + 1],
                        op0=mybir.AluOpType.subtract,
                        op1=mybir.AluOpType.mult,
                    )

            def a_ln(x_sb, split=False):
                xn_sb = xnpool.tile([P, G, D_MODEL], BF16, tag="xn")
                if split:
                    a_ln_half(x_sb, xn_sb, 0, "a")
                    a_ln_half(x_sb, xn_sb, 1, "b")
                else:
                    # single chain over all G (fewer small ops)
                    stats = statp.tile([P, G, 2, 6], F32, tag="bnst")
                    mv = statp.tile([P, G, 2], F32, tag="mv")
                    for g in range(G):
                        nc.vector.bn_stats(stats[:, g, 0, :], x_sb[:, g, 0:512])
                        nc.vector.bn_stats(stats[:, g, 1, :], x_sb[:, g, 512:1024])
                        nc.vector.bn_aggr(mv[:, g, :], stats[:, g])
                    vp = statp.tile([P, G], F32, tag="vp")
                    nc.vector.tensor_scalar(
                        out=vp,
                        in0=mv[:, :, 1],
                        scalar1=EPS,
                        scalar2=1.0 / (S_X * S_X),
                        op0=mybir.AluOpType.add,
                        op1=mybir.AluOpType.mult,
                    )
                    rs = statp.tile([P, G], F32, tag="rs")
                    nc.vector.tensor_scalar(
                        out=rs.bitcast(I32),
                        in0=vp.bitcast(I32),
                        scalar1=1,
                        scalar2=None,
                        op0=mybir.AluOpType.arith_shift_right,
                    )
                    nc.vector.tensor_scalar(
                        out=rs.bitcast(I32),
                        in0=rs.bitcast(I32),
                        scalar1=-1,
                        scalar2=MAGIC,
                        op0=mybir.AluOpType.mult,
                        op1=mybir.AluOpType.add,
                    )
                    sq = statp.tile([P, G], F32, tag="sq")
                    for _ in range(2):
                        nc.vector.tensor_tensor(
                            out=sq, in0=rs, in1=rs, op=mybir.AluOpType.mult
                        )
                        nc.vector.tensor_tensor(
                            out=sq, in0=sq, in1=vp, op=mybir.AluOpType.mult
                        )
                        nc.vector.tensor_scalar(
                            out=sq,
                            in0=sq,
                            scalar1=-0.5,
                            scalar2=1.5,
                            op0=mybir.AluOpType.mult,
                            op1=mybir.AluOpType.add,
                        )
                        nc.vector.tensor_tensor(
                            out=rs, in0=rs, in1=sq, op=mybir.AluOpType.mult
                        )
                    for g in range(G):
                        nc.vector.tensor_scalar(
                            out=xn_sb[:, g, :],
                            in0=x_sb[:, g, :],
                            scalar1=mv[:, g, 0:1],
                            scalar2=rs[:, g : g + 1],
                            op0=mybir.AluOpType.subtract,
                            op1=mybir.AluOpType.mult,
                        )
                return xn_sb

            def tr_alloc():
                xtb = xtpool.tile([P, KB, G, P], BF16, tag="xtb")
                xt8 = xtpool.tile([P, KF + 1, G, P], FP8, tag="xt8")
                if KF % 2:
                    # dead slot pairs with chunk 7 in the last DR matmul
                    # (zero weights); gpsimd is otherwise idle
                    nc.gpsimd.memset(xt8[:, KF : KF + 1], 0)
                return xtb, xt8

            def tr_quarter(xt, xn_sb, r, half):
                """transposes for pair r (k-chunks 2r, 2r+1), g-half."""
                xtb, xt8, ptrs = xt
                if half == 0:
                    ptr = ps_tr.tile([P, 2, G, P], BF16, tag="ptr")
                    ptrs[r] = ptr
                ptr = ptrs[r]
                gs = range(half * (G // 2), (half + 1) * (G // 2))
                for kk in range(2):
                    k = 2 * r + kk
                    for g in gs:
                        nc.tensor.transpose(
                            ptr[:, kk, g, :],
                            xn_sb[:, g, k * P : (k + 1) * P],
                            ident,
                        )

            def tr_cast(xt, r, half=None):
                """ACT copyback for pair r (optionally one g-half).
                Chunk c goes to xtb[c] (c < KB) or xt8[c-KB]; pair 1
                straddles the split so it issues two half-width copies.
                The dead slot xt8[KF] is filled with chunk-7 data (its
                weights are zero)."""
                xtb, xt8, ptrs = xt
                ptr = ptrs[r]
                if half is None:
                    gsl = slice(0, G)
                else:
                    gsl = slice(half * (G // 2), (half + 1) * (G // 2))
                c0 = 2 * r
                if c0 + 1 < KB:
                    nc.scalar.activation(
                        out=xtb[:, c0 : c0 + 2, gsl],
                        in_=ptr[:, :, gsl],
                        func=mybir.ActivationFunctionType.Copy,
                    )
                elif c0 >= KB:
                    nc.scalar.activation(
                        out=xt8[:, c0 - KB : c0 - KB + 2, gsl],
                        in_=ptr[:, :, gsl],
                        func=mybir.ActivationFunctionType.Copy,
                    )
                else:
                    nc.scalar.activation(
                        out=xtb[:, c0 : c0 + 1, gsl],
                        in_=ptr[:, 0:1, gsl],
                        func=mybir.ActivationFunctionType.Copy,
                    )
                    nc.scalar.activation(
                        out=xt8[:, 0:1, gsl],
                        in_=ptr[:, 1:2, gsl],
                        func=mybir.ActivationFunctionType.Copy,
                    )

            def a_tr_all(xn_sb, split=False):
                """full transpose+cast for one tile (startup path)"""
                xtb, xt8 = tr_alloc()
                xt = (xtb, xt8, {})
                for r in range(KD // 2):
                    if split:
                        tr_quarter(xt, xn_sb, r, 0)
                        tr_cast(xt, r, 0)
                        tr_quarter(xt, xn_sb, r, 1)
                        tr_cast(xt, r, 1)
                    else:
                        tr_quarter(xt, xn_sb, r, 0)
                        tr_quarter(xt, xn_sb, r, 1)
                        tr_cast(xt, r)
                return xtb, xt8

            # software pipeline: x-DMA two tiles ahead, LayerNorm one tile
            # ahead, transposes one tile ahead interleaved into the m3
            # chunk loop
            xn_cur = a_ln(x_tiles[0], split=True)
            xt_cur = a_tr_all(xn_cur, split=True)
            xn_next = a_ln(x_tiles[1])
            for i, (off, T) in enumerate(tiles):
                x_sb = x_tiles[i]
                xtb, xt8 = xt_cur
                if i + 2 < len(tiles):
                    x_tiles.append(a_dma(i + 2))
                # the final overlapped tile recomputes only its genuinely
                # new tokens (the trailing g-groups) through the matmuls;
                # LN/transpose of the overlap region is off critical path
                is_ov = i == len(tiles) - 1 and TOK % TILE_T != 0
                g_lo = (TILE_T - TOK % TILE_T) // P if is_ov else 0
                nT = T - g_lo * P
                # cs9 allocated + ones row DMA'd early (row 8 is only
                # reachable by DMA; issuing here hides its latency)
                cs9 = statp.tile([D_STATE + 1, TILE_T], F32R, tag="cs9")
                nc.sync.dma_start(
                    cs9[D_STATE : D_STATE + 1, :nT], ones[:, :nT]
                )
                # matmul 1: K-split, common (x*S_X)(W*S_W) scaling in PSUM;
                # SiLU per m-chunk with the 1/(S_W*S_X) descale fused
                projT = projp.tile([P, ME, TILE_T], BF16, tag="projT")
                for m in range(ME):
                    p1 = ps1.tile([P, TILE_T], F32, tag="p1")
                    if has_bias:
                        nc.tensor.matmul(
                            p1[:, :nT],
                            lhsT=bp_sb[:, m * P : (m + 1) * P],
                            rhs=ones_sb[:, :nT],
                            start=True,
                            stop=False,
                            skip_group_check=True,
                        )
                    for k in range(KB):
                        nc.tensor.matmul(
                            p1[:, :nT],
                            lhsT=wbt_sb[:, k, m * P : (m + 1) * P],
                            rhs=xtb[:, k, g_lo:, :],
                            start=(k == 0 and not has_bias),
                            stop=False,
                            skip_group_check=has_bias,
                        )
                    for r in range(KP):
                        nc.tensor.matmul(
                            p1[:, :nT],
                            lhsT=wp8_sb[:, r, :, m * P : (m + 1) * P],
                            rhs=xt8[:, 2 * r : 2 * r + 2, g_lo:, :],
                            start=False,
                            stop=(r == KP - 1),
                            perf_mode=mybir.MatmulPerfMode.DoubleRow,
                            skip_group_check=has_bias,
                        )
                    nc.scalar.activation(
                        out=projT[:, m, :nT],
                        in_=p1[:, :nT],
                        func=mybir.ActivationFunctionType.Silu,
                        bias=0.0,
                        scale=1.0 / (S_W * S_X),
                    )
                # matmul 2: bf16, [D_STATE, T]
                p2 = ps2.tile([D_STATE, TILE_T], F32, tag="p2")
                for k2 in range(ME):
                    nc.tensor.matmul(
                        p2[:, :nT],
                        lhsT=wst_sb[:, k2, :],
                        rhs=projT[:, k2, :nT],
                        start=(k2 == 0),
                        stop=(k2 == ME - 1),
                    )
                # silu2 emitted BEFORE the next tile's transpose casts so
                # matmul 3 never queues behind them on the ACT engine
                nc.scalar.activation(
                    out=cs9[:D_STATE, :nT],
                    in_=p2[:, :nT],
                    func=mybir.ActivationFunctionType.Silu,
                    bias=b2_sb,
                    scale=1.0,
                )
                # matmul 3 in [P,512] chunks (double-buffered over two
                # 1-bank PSUM slots), with the next tile's transposes
                # interleaved so the PE fills the DVE-residual drain time;
                # residual add fused into the DVE copyback
                do_tr = i + 1 < len(tiles)
                if do_tr:
                    xtb_n, xt8_n = tr_alloc()
                    xt_n = (xtb_n, xt8_n, {})
                out_sb = outp.tile([P, G, D_MODEL], F32, tag="out")
                nch = (G - g_lo) * 2
                for c in range(nch):
                    g = g_lo + c // 2
                    h = c % 2
                    p3 = ps3.tile([P, 512], F32, tag="p3")
                    nc.tensor.matmul(
                        p3[:, :],
                        lhsT=cs9[:, (g - g_lo) * P : (g - g_lo + 1) * P],
                        rhs=wo9_sb[:, h * 512 : (h + 1) * 512],
                        start=True,
                        stop=True,
                    )
                    # interleave one transpose quarter per chunk
                    if do_tr and c < 8:
                        r, half = c // 2, c % 2
                        tr_quarter(xt_n, xn_next, r, half)
                        if half == 1:
                            tr_cast(xt_n, r)
                    nc.vector.tensor_add(
                        out=out_sb[:, g, h * 512 : (h + 1) * 512],
                        in0=p3[:, :],
                        in1=x_sb[:, g, h * 512 : (h + 1) * 512],
                    )
                if do_tr:
                    # finish any transpose quarters not covered (nch < 8)
                    for c in range(nch, 8):
                        r, half = c // 2, c % 2
                        tr_quarter(xt_n, xn_next, r, half)
                        if half == 1:
                            tr_cast(xt_n, r)
                    xt_cur = (xtb_n, xt8_n)
                nc.sync.dma_start(
                    y[off + g_lo * P : off + T, :].rearrange(
                        "(g p) d -> p g d", p=P
                    ),
                    out_sb[:, g_lo:, :],
                )
                # LN for the tile after is emitted BEHIND this tile's
                # residual adds: the DVE queue is in-order, and parking
                # ~10us of LN work ahead of the resid TTs would stall
                # matmul 3 on the ps3 rotation
                if i + 2 < len(tiles):
                    xn_next = a_ln(x_tiles[i + 2])

    _split_multi_waits(nc)
    return nc


_NC_CACHE = {}


def _get_nc(has_bias: bool):
    if has_bias not in _NC_CACHE:
        _NC_CACHE[has_bias] = build_kernel(has_bias)
    return _NC_CACHE[has_bias]


def make_in_maps(inputs):
    x = np.ascontiguousarray(inputs["x"], dtype=np.float32).reshape(-1, D_MODEL)
    W_proj = np.asarray(inputs["W_proj"], dtype=np.float64)
    b_proj = np.asarray(inputs["b_proj"], dtype=np.float64)
    W_state = np.asarray(inputs["W_state"], dtype=np.float32)
    b_state = np.asarray(inputs["b_state"], dtype=np.float32)
    W_out = np.asarray(inputs["W_out"], dtype=np.float32)
    b_out = np.asarray(inputs["b_out"], dtype=np.float32)
    initial_state = np.asarray(inputs["initial_state"], dtype=np.float32)
    gamma = np.asarray(inputs["gamma"], dtype=np.float64)
    beta = np.asarray(inputs["beta"], dtype=np.float64)

    # fold the LayerNorm affine into the projection
    Wp = W_proj * gamma[None, :]  # [d_inner, d_model]
    bp = b_proj + W_proj @ beta  # [d_inner]
    has_bias = bool(np.any(bp != 0.0))

    WpT = np.ascontiguousarray(Wp.T) * S_W  # [d_model, d_inner], scaled
    # k-chunks 0..3 in bf16
    wbt = WpT[: KB * P].astype(ml_dtypes.bfloat16)
    # k-chunks KB..7 in fp8e4, DoubleRow pair layout [p, r, j, e]; the
    # last pair's second slot is zero weights (dead xt8 slot)
    w8 = np.clip(WpT[KB * P :], -224.0, 224.0).astype(ml_dtypes.float8_e4m3)
    w8full = np.zeros((KP * 2, P, D_INNER), dtype=ml_dtypes.float8_e4m3)
    w8full[:KF] = w8.reshape(KF, P, D_INNER)
    wp8 = np.ascontiguousarray(
        w8full.reshape(KP, 2, P, D_INNER).transpose(2, 0, 1, 3)
    )

    shared = {
        "wbt": np.ascontiguousarray(wbt),
        "wp8": wp8,
        "wst": np.ascontiguousarray(W_state.T.astype(ml_dtypes.bfloat16)),
        "wo9": np.ascontiguousarray(
            np.concatenate([W_out.T, b_out[None, :]], axis=0)
        ),
        "bp": np.ascontiguousarray((bp * S_W * S_X).astype(np.float32))[None, :],
        "b2": np.ascontiguousarray(
            (b_state + initial_state.reshape(-1)).reshape(D_STATE, 1)
        ),
        "ones": np.ones((1, TILE_T), dtype=np.float32),
        "ident": np.eye(P, dtype=ml_dtypes.bfloat16),
    }
    in_maps = []
    for c in range(N_CORES):
        m = {"x": np.ascontiguousarray(x[c * TOK : (c + 1) * TOK])}
        m.update(shared)
        in_maps.append(m)
    return in_maps, has_bias


def kernel(**inputs) -> np.ndarray:
    in_maps, has_bias = make_in_maps(inputs)
    nc = _get_nc(has_bias)
    res = run_bass_kernel_spmd(nc, in_maps, core_ids=list(range(N_CORES)))
    out = np.concatenate([res.results[c]["y"] for c in range(N_CORES)], axis=0)
    return out.reshape(np.asarray(inputs["x"]).shape)
